# revision 1
# baseline (speedup 1.0000x reference)
"""Trainium2 Bass kernel for the modified-MDPN dendrite model.

Math per output element (b, i, j, m):
    acc = sum_r log(prod_c u)   with u = atan(10*(x*w - q))/pi + 1.1
        = log(prod_{r,c} u)                     (u > 0 always)
Then 4x4 spatial maxpool, flatten (i_o, j_o, m), fc1(7744->128)+relu,
fc2(128->10).

Device strategy (8 NeuronCores, data parallel over batch, 2 images/core):
  - partitions p = m*8 + cp (m: 16 filters, cp: 8 chunks of 12 output rows;
    12 = 3 pool groups so the 4x4 maxpool never crosses partitions; output
    rows 88..95 are garbage lanes masked later by zero fc1 weights; x is
    zero-padded to 104 rows on host for the halo).
  - per window tap (r, c) of the 81: one ACT Arctan instruction over
    [128, 2*12*88] with per-partition scale=10w, bias=-10q folded into the
    activation's pre-affine (HW arctan is accurate far beyond +-pi/2 -- the
    CoreSim range assert is conservative); one DVE tensor_scalar
    (u = t/pi + 1.1, bf16, 4x mode); one DVE multiply into a running
    product (bf16, 2x mode, ping-pong buffers).
  - ln is monotonic, so the 4x4 maxpool runs on the bf16 *products* (two
    free-dim max-reduces, entirely within partitions thanks to the 12-row
    chunking) and Ln runs on the 16x smaller pooled map. ln of the 81-tap
    product equals the reference's sum-over-rows-of-log-of-column-products
    (u > 0; product stays within e^-42..e^+39, inside bf16/f32 range).
  - the last tap runs per-image so image 0's pool/ln chain overlaps
    image 1's final atan on ACT; tap 0 is split by rows so the first atan
    only waits for the first ~0.25MB of the input DMA.
  - fc1: the pooled map y2 is already [(m,cp) partitions, (b,f2)], so the
    7744-long contraction runs directly as 66 accumulating K=128 matmuls
    (rhs strided over b) against host-permuted, zero-padded bf16 fc1
    weights -- no transpose needed; relu+bias fused on ACT; fc2 is one
    matmul.

Engine balance per core (cost model, validated against HW wall-clock
slope with an on-device repeat loop: ~160us measured per 81-tap body vs
159.7us modeled): ACT ~157us stream (critical path), DVE ~146us,
PE ~7us; total ~176us = startup/exit-barrier + ACT stream + ~3us tail.
Pure atan streaming floor is 142.6us; the gap is the per-instruction
SBUF turnaround (~15us, silicon errata) and the 12-row padding (~12us)
that buys the in-partition pool. Measured dead ends: GPSIMD z-precompute
to batch taps per ACT instruction (solo rate 1.09 cyc/elem but ~12x
degradation under concurrent DVE load from SBUF-port contention),
tap-parity instruction packing (cross-partition product combine costs
more than the amortized bubbles), PSUM activation outputs (drop DVE
perf modes).
"""

import sys

sys.path.insert(0, "/opt/trn_rl_repo")

import ml_dtypes
import numpy as np

import concourse.bacc as bacc
import concourse.mybir as mybir
from concourse import tile
from concourse.bass_utils import run_bass_kernel_spmd

AFT = mybir.ActivationFunctionType
ALU = mybir.AluOpType
F32 = mybir.dt.float32
BF16 = mybir.dt.bfloat16

M = 16          # filters
N = 9           # window side
IMG = 96
S = 88          # sliding-window output side
SP = 22         # pooled side
B = 16          # global batch
NCORES = 8
BL = B // NCORES          # images per core (2)
CP = 8                    # row chunks per image
RP = 12                   # output rows per chunk (12*8 = 96 >= 88)
GI = RP // 4              # pooled row-groups per chunk (3)
HALO = RP + N - 1         # input rows per chunk (20)
ROWS_PAD = RP * (CP - 1) + HALO   # padded input rows (104)
FD = BL * RP * S          # free elems per tap instruction (2112)
F2 = GI * SP              # pooled positions per (partition, image) (66)
PI = float(np.pi)

_CACHE = {}


def _build_nc():
    nc = bacc.Bacc("TRN2", target_bir_lowering=False, debug=False)

    xp = nc.declare_dram_parameter("xp", [128, BL * HALO * IMG], F32, isOutput=False)
    ws = nc.declare_dram_parameter("ws", [128, 81], F32, isOutput=False)
    qs = nc.declare_dram_parameter("qs", [128, 81], F32, isOutput=False)
    w1 = nc.declare_dram_parameter("w1", [128, F2 * 128], BF16, isOutput=False)
    w2 = nc.declare_dram_parameter("w2", [128, 10], F32, isOutput=False)
    b1 = nc.declare_dram_parameter("b1", [128, 1], F32, isOutput=False)
    b2 = nc.declare_dram_parameter("b2", [10, 1], F32, isOutput=False)
    out = nc.declare_dram_parameter("out", [10, BL], F32, isOutput=True)

    with tile.TileContext(nc) as tc:
        with (
            tc.tile_pool(name="consts", bufs=1) as cpool,
            tc.tile_pool(name="work", bufs=3) as wpool,
            tc.tile_pool(name="state", bufs=1) as spool,
            tc.tile_pool(name="psum", bufs=1, space="PSUM") as ppool,
        ):
            xs = cpool.tile([128, BL * HALO * IMG], F32, tag="xs")
            wst = cpool.tile([128, 81], F32, tag="wst")
            qst = cpool.tile([128, 81], F32, tag="qst")
            w1t = cpool.tile([128, F2 * 128], BF16, tag="w1t")
            w2t = cpool.tile([128, 10], F32, tag="w2t")
            b1t = cpool.tile([128, 1], F32, tag="b1t")
            b2t = cpool.tile([10, 1], F32, tag="b2t")

            # DMA order matters: the first atan waits on wst/qst and the
            # r=0 halo rows, so issue the small tensors first, then the
            # first 12 halo rows, then the rest; the big fc1 weights last.
            nc.sync.dma_start(wst[:], ws[:])
            nc.sync.dma_start(qst[:], qs[:])
            xsr = xs[:].rearrange("p (b il j) -> p b il j", b=BL, il=HALO, j=IMG)
            xpr = xp.rearrange("p (b il j) -> p b il j", b=BL, il=HALO, j=IMG)
            for p0 in range(0, 128, 32):
                nc.sync.dma_start(
                    xsr[p0 : p0 + 32, :, 0:6], xpr[p0 : p0 + 32, :, 0:6]
                )
            for p0 in range(0, 128, 32):
                nc.sync.dma_start(
                    xsr[p0 : p0 + 32, :, 6:RP], xpr[p0 : p0 + 32, :, 6:RP]
                )
            for p0 in range(0, 128, 64):
                nc.sync.dma_start(
                    xsr[p0 : p0 + 64, :, RP:HALO], xpr[p0 : p0 + 64, :, RP:HALO]
                )
            nc.sync.dma_start(b1t[:], b1[:])
            nc.sync.dma_start(b2t[:], b2[:])
            nc.sync.dma_start(w2t[:], w2[:])
            nc.sync.dma_start(w1t[:], w1[:])

            xr = xs[:].rearrange("p (b il j) -> p b il j", b=BL, il=HALO, j=IMG)

            rp_tiles = [
                spool.tile([128, FD], BF16, tag="rp0", name="rp0"),
                spool.tile([128, FD], BF16, tag="rp1", name="rp1"),
            ]
            cur = 0
            NSPLIT = 80      # taps [NSPLIT, 81) run per-image for tail overlap
            for t in range(NSPLIT):
                r, c = divmod(t, N)
                xv = xr[:, :, r : r + RP, c : c + S]
                ut = wpool.tile([128, BL, RP, S], BF16, tag="atan")
                if t == 0:
                    # split tap 0 by rows so the first atan only waits for
                    # the first 6 halo rows of the input DMA
                    nc.scalar.activation(
                        ut[:, :, 0:6], xv[:, :, 0:6], AFT.Arctan,
                        bias=qst[:, t : t + 1], scale=wst[:, t : t + 1],
                    )
                    nc.scalar.activation(
                        ut[:, :, 6:RP], xv[:, :, 6:RP], AFT.Arctan,
                        bias=qst[:, t : t + 1], scale=wst[:, t : t + 1],
                    )
                else:
                    nc.scalar.activation(
                        ut[:], xv, AFT.Arctan,
                        bias=qst[:, t : t + 1], scale=wst[:, t : t + 1],
                    )
                uf = ut[:].rearrange("p b il j -> p (b il j)")
                if t == 0:
                    nc.vector.tensor_scalar(
                        rp_tiles[0][:], uf, 1.0 / PI, 1.1, ALU.mult, ALU.add
                    )
                else:
                    un = wpool.tile([128, FD], BF16, tag="un")
                    nc.vector.tensor_scalar(
                        un[:], uf, 1.0 / PI, 1.1, ALU.mult, ALU.add
                    )
                    nxt = 1 - cur
                    nc.vector.tensor_tensor(
                        rp_tiles[nxt][:], rp_tiles[cur][:], un[:], ALU.mult
                    )
                    cur = nxt

            # Last taps run per-image so image b's pool/ln/transpose chain
            # overlaps the other image's remaining atans on ACT.
            # ln is monotonic, so maxpool the bf16 products first and take
            # Ln on the 16x smaller pooled map (saves ACT time and takes
            # Ln off the serial tail).
            FD1 = RP * S
            shared = rp_tiles[cur][:].rearrange("p (b f) -> p b f", b=BL, f=FD1)
            y2u = spool.tile([128, BL * F2], BF16, tag="y2u")
            y2uv = y2u[:].rearrange("p (b f2) -> p b f2", b=BL, f2=F2)
            y2 = spool.tile([128, BL * F2], BF16, tag="y2")
            y2v = y2[:].rearrange("p (b f2) -> p b f2", b=BL, f2=F2)
            p1 = spool.tile([128, BL * RP * SP], BF16, tag="p1")
            p1bv = p1[:].rearrange("p (b f) -> p b f", b=BL, f=RP * SP)
            for b in range(BL):
                rpb = [
                    spool.tile([128, FD1], BF16, tag=f"rpb{b}{i}", name=f"rpb{b}{i}")
                    for i in range(2)
                ]
                bcur = -1          # -1 means "shared tile half"
                for t in range(NSPLIT, 81):
                    r, c = divmod(t, N)
                    xvb = xr[:, b : b + 1, r : r + RP, c : c + S]
                    utb = wpool.tile([128, 1, RP, S], BF16, tag="atanb")
                    nc.scalar.activation(
                        utb[:], xvb, AFT.Arctan,
                        bias=qst[:, t : t + 1], scale=wst[:, t : t + 1],
                    )
                    ufb = utb[:].rearrange("p b il j -> p (b il j)")
                    unb = wpool.tile([128, FD1], BF16, tag="unb")
                    nc.vector.tensor_scalar(
                        unb[:], ufb, 1.0 / PI, 1.1, ALU.mult, ALU.add
                    )
                    src = shared[:, b] if bcur < 0 else rpb[bcur][:]
                    bnxt = (bcur + 1) % 2
                    nc.vector.tensor_tensor(rpb[bnxt][:], src, unb[:], ALU.mult)
                    bcur = bnxt
                final_b = shared[:, b] if bcur < 0 else rpb[bcur][:]

                # maxpool over j (groups of 4), output laid out (f2, ii)
                # with f2 = ig*22 + jo, il = 4*ig + ii
                accv = final_b.rearrange(
                    "p (il jo jj) -> p il jo jj", il=RP, jo=SP, jj=4
                )
                p1w = p1bv[:, b].rearrange(
                    "p (ig jo ii) -> p ig ii jo", ig=GI, jo=SP, ii=4
                )
                nc.vector.tensor_reduce(p1w, accv, mybir.AxisListType.X, ALU.max)

                # maxpool over i (= groups of 4 rows: ii axis, innermost)
                p1i = p1bv[:, b].rearrange("p (f2 ii) -> p f2 ii", f2=F2, ii=4)
                nc.vector.tensor_reduce(
                    y2uv[:, b], p1i, mybir.AxisListType.X, ALU.max
                )

                # dendrite output: ln of the pooled 81-tap product
                nc.scalar.activation(y2v[:, b], y2uv[:, b], AFT.Ln)

            # fc1: y2 is already [(m, cp) partitions, (b, f2)] -- contract
            # the partition dim directly: 66 accumulating matmuls of K=128,
            # rhs strided over b. No transpose needed.
            ph = ppool.tile([128, BL], F32, tag="ph")
            y2f = y2[:].rearrange("p (b f2) -> p f2 b", b=BL, f2=F2)
            for g in range(F2):
                nc.tensor.matmul(
                    ph[:],
                    w1t[:, g * 128 : (g + 1) * 128],
                    y2f[:, g],
                    start=(g == 0),
                    stop=(g == F2 - 1),
                )
            h = spool.tile([128, BL], F32, tag="h")
            nc.scalar.activation(h[:], ph[:], AFT.Relu, bias=b1t[:, 0:1])

            # fc2
            po = ppool.tile([10, BL], F32, tag="po")
            nc.tensor.matmul(po[:], w2t[:, 0:10], h[:], start=True, stop=True)
            osb = spool.tile([10, BL], F32, tag="osb")
            nc.scalar.activation(osb[:], po[:], AFT.Identity, bias=b2t[:, 0:1])
            nc.sync.dma_start(out[:], osb[:])

    nc.compile()
    return nc


def _prep_inputs(x, w, q, fc1_w, fc1_b, fc2_w, fc2_b):
    x = np.asarray(x, np.float32)
    w = np.asarray(w, np.float32)
    q = np.asarray(q, np.float32)
    fc1_w = np.asarray(fc1_w, np.float32)
    fc1_b = np.asarray(fc1_b, np.float32)
    fc2_w = np.asarray(fc2_w, np.float32)
    fc2_b = np.asarray(fc2_b, np.float32)

    xpad = np.zeros((B, ROWS_PAD, IMG), np.float32)
    xpad[:, :IMG, :] = x
    # halo chunks: [B, CP, HALO, IMG]
    xh = np.stack(
        [xpad[:, RP * cp : RP * cp + HALO, :] for cp in range(CP)], axis=1
    )

    ws = np.repeat(10.0 * w.reshape(M, 81), CP, axis=0)          # [128, 81]
    qs = np.repeat(-10.0 * q.reshape(M, 81), CP, axis=0)

    # fc1 weights: w1[f2, (m*8+cp)*128 + n] = fc1_w[n, io*352 + jo*16 + m]
    # with io = 3*cp + ig, f2 = ig*22 + jo; zero where io >= 22 (the
    # garbage pool lanes from the 12-row chunking).
    fw = fc1_w.reshape(128, SP, SP, M)            # [n, io, jo, m]
    a = fw.transpose(1, 2, 3, 0)                  # [io, jo, m, n]
    io_idx = 3 * np.arange(CP)[:, None] + np.arange(GI)[None, :]   # [cp, ig]
    valid = (io_idx < SP).astype(np.float32)
    b6 = a[np.clip(io_idx, 0, SP - 1)]            # [cp, ig, jo, m, n]
    b6 = b6 * valid[:, :, None, None, None]
    # [cp, ig, jo, m, n] -> [(m, cp), (ig, jo), n]
    w1 = np.ascontiguousarray(
        b6.transpose(3, 0, 1, 2, 4).reshape(128, F2 * 128)
    ).astype(ml_dtypes.bfloat16)

    w2 = np.ascontiguousarray(fc2_w.T)            # [128, 10]
    b1 = fc1_b.reshape(128, 1).astype(np.float32)
    b2 = fc2_b.reshape(10, 1).astype(np.float32)

    in_maps = []
    for k in range(NCORES):
        arr = xh[BL * k : BL * k + BL]            # [BL, CP, HALO, IMG]
        xpk = np.broadcast_to(arr[None], (M, BL, CP, HALO, IMG))
        xpk = np.ascontiguousarray(
            xpk.transpose(0, 2, 1, 3, 4).reshape(128, BL * HALO * IMG)
        )
        in_maps.append(
            dict(xp=xpk, ws=ws, qs=qs, w1=w1, w2=w2, b1=b1, b2=b2)
        )
    return in_maps


def kernel(x, w, q, fc1_w, fc1_b, fc2_w, fc2_b):
    if "nc" not in _CACHE:
        _CACHE["nc"] = _build_nc()
    nc = _CACHE["nc"]
    in_maps = _prep_inputs(x, w, q, fc1_w, fc1_b, fc2_w, fc2_b)
    # The axon-tunneled devices occasionally throw a transient
    # NRT_EXEC_UNIT_UNRECOVERABLE on the first execution of a fresh NEFF;
    # a retry has always succeeded with identical results.
    last_err = None
    for attempt in range(3):
        try:
            res = run_bass_kernel_spmd(nc, in_maps, list(range(NCORES)))
            break
        except Exception as e:  # noqa: BLE001 - retry transient device faults
            last_err = e
            import time as _time
            _time.sleep(5 * (attempt + 1))
    else:
        raise last_err
    _CACHE["last_exec_time_ns"] = res.exec_time_ns
    _CACHE["last_results"] = res
    outp = np.empty((B, 10), np.float32)
    for k in range(NCORES):
        o = np.asarray(res.results[k]["out"], np.float32)   # [10, BL]
        outp[BL * k : BL * k + BL, :] = o.T
    return outp

